# revision 38
# baseline (speedup 1.0000x reference)
"""Distributed causal multi-head attention for Trainium2 (8 NeuronCores).

Problem: B=2, S=2048, NX=1024, H=16 heads, D=64.
  qkv = x @ w_attn + b_attn ; q,k,v split; causal softmax(q k^T / 8) v ; @ w_proj + b_proj

Sharding: pure tensor-parallel by head. Core s owns global heads {2s, 2s+1}
for BOTH batches (QKV column-split). After attention, two AllToAlls (one per
local head) reshard heads->sequence: every core ends up with the full hidden
dim for its own 512 output rows (core c -> batch c//4, rows 512*(c%4)), so
c_proj contracts over the REAL 1024 hidden dim - no zero-padded halves, and
each A2A carries 512KB with all 8 slots distinct.

Layout: host passes x pre-tiled/transposed so QKV projections, scores and PV
products all run in matmul-native layouts with zero on-chip transposes, and
every weight/x load is a single contiguous DMA trigger (the SP queue paced
the old startup). Scores are computed transposed ([k, q]) with per-head kT
tiles whose other-head rows are zeroed so score matmuls run at K=128 (the
zeros annihilate the foreign q rows; K=64 empirically triggers much harsher
HAM clock throttling). The softmax reduction over k lands on the partition
axis, where a ones-column inside the v tiles yields the denominator for free
in row 64 of the same PV matmul; exp() needs no max-subtraction (scores are
bounded). ScalarE exp (~83us over 96 up-to-1024-wide calls) is the critical
engine, so attention is pair-fused per head: one key-block loop computes
scores+exp for BOTH batches back-to-back while PV runs live for b0 only;
b1's ex tiles park in SBUF and its per-chunk PV runs as pure-PE bursts
mid-loop the moment the b0 accumulator bank for that chunk frees, so each
head's A2A fires a few us after its last exp. c_proj pass 0 (head-0 blocks)
executes during the second A2A's flight. Matmul operands are bf16;
accumulation stays fp32 in PSUM.
"""

import sys

sys.path.insert(0, "/opt/trn_rl_repo")

import numpy as np
import ml_dtypes

BF16 = ml_dtypes.bfloat16

B = 2
S = 2048
NX = 1024
H = 16
D = 64
HL = 2           # heads per core (tensor-parallel 8-way)
P = 128
SC = 512         # output chunk (A2A slot granularity)
NQC = S // SC    # 4 chunks per (head, batch) instance
NE = NX // P     # 8 contraction tiles
NKB = S // P     # 16 key blocks
WQ = 1024        # max score-tile width

_COMPILED = None


def _build():
    import concourse.bass as bass  # noqa: F401
    import concourse.mybir as mybir
    import concourse.tile as tile
    from concourse import bacc

    f32 = mybir.dt.float32
    f32r = mybir.dt.float32r
    bf16 = mybir.dt.bfloat16
    Identity = mybir.ActivationFunctionType.Identity
    Exp = mybir.ActivationFunctionType.Exp

    nc = bacc.Bacc("TRN2", target_bir_lowering=False, debug=False, num_devices=8)

    # host-pretiled inputs (single contiguous DMA each)
    xtt = nc.dram_tensor("xtt", [2 * NQC, P, NE * SC], bf16, kind="ExternalInput")
    wqk = nc.dram_tensor("wqk", [P, NE * 2 * P], bf16, kind="ExternalInput")
    wv = nc.dram_tensor("wv", [P, NE * P], bf16, kind="ExternalInput")
    wp = nc.dram_tensor("wp", [P, 8 * NX], bf16, kind="ExternalInput")
    bqk = nc.dram_tensor("bqk", [P, 2], f32, kind="ExternalInput")
    bp16 = nc.dram_tensor("bp16", [1, NX], bf16, kind="ExternalInput")
    idcz = nc.dram_tensor("idcz", [P, 2 * P], bf16, kind="ExternalInput")
    ones1 = nc.dram_tensor("ones1", [1, P], f32, kind="ExternalInput")
    ones1b = nc.dram_tensor("ones1b", [1, P], bf16, kind="ExternalInput")
    out_ext = nc.dram_tensor("out", [SC, NX], f32, kind="ExternalOutput")

    with tile.TileContext(nc) as tc:
        with (
            tc.tile_pool(name="const", bufs=1) as const_pool,
            tc.tile_pool(name="xt", bufs=1) as xt_pool,
            tc.tile_pool(name="w", bufs=1) as w_pool,
            tc.tile_pool(name="qk", bufs=1) as qk_pool,
            tc.tile_pool(name="vsb", bufs=1) as v_pool,
            tc.tile_pool(name="lh", bufs=1) as lh_pool,
            tc.tile_pool(name="exp", bufs=4) as exp_pool,
            tc.tile_pool(name="exs", bufs=1) as exs_pool,
            tc.tile_pool(name="osb", bufs=2) as osb_pool,
            tc.tile_pool(name="small", bufs=2) as small_pool,
            tc.tile_pool(name="wide", bufs=2, space="PSUM") as wide_ps,
            tc.tile_pool(name="atps", bufs=4, space="PSUM") as at_ps_pool,
            tc.tile_pool(name="dram", bufs=1, space="DRAM") as dram_pool,
        ):
            # ---- weight + x loads, first-needed first ----
            wqk_sb = w_pool.tile([P, NE * 2 * P], bf16, name="wqk_sb")
            nc.sync.dma_start(wqk_sb[:], wqk[:])
            xt_sb = {}
            for sc in range(2 * NQC):
                t = xt_pool.tile([P, NE * SC], bf16, name=f"xt{sc}", tag=f"xt{sc}")
                nc.sync.dma_start(t[:], xtt[sc])
                xt_sb[sc] = t
                if sc == 0:
                    wv_sb = w_pool.tile([P, NE * P], bf16, name="wv_sb")
                    nc.sync.dma_start(wv_sb[:], wv[:])

            # ---- constants ----
            bqk_sb = const_pool.tile([P, 2], f32, name="bqk_sb")
            nc.sync.dma_start(bqk_sb[:], bqk[:])
            idcz_sb = const_pool.tile([P, 2 * P], bf16, name="idcz_sb")
            nc.sync.dma_start(idcz_sb[:], idcz[:])
            id_sb = idcz_sb[:, 0:P]
            czb_sb = idcz_sb[:, P : 2 * P]
            ones1_sb = const_pool.tile([1, P], f32r, name="ones1_sb")
            nc.sync.dma_start(ones1_sb[:], ones1[:].bitcast(f32r))
            ones1b_sb = const_pool.tile([1, P], bf16, name="ones1b_sb")
            nc.sync.dma_start(ones1b_sb[:], ones1b[:])
            bp_sb = const_pool.tile([1, NX], bf16, name="bp_sb")
            nc.sync.dma_start(bp_sb[:], bp16[:])

            # ---- phase 1: projections ----
            # per-chunk q tiles [128 feats (2 heads), 512] and per-(head,
            # chunk) ktz tiles with the other head's rows zeroed, so score
            # matmuls run at K=128 (zeros annihilate the foreign q rows) -
            # empirically this keeps the HAM clock gate at full rate where
            # K=64 throttles. Chunk granularity keeps dependencies fine:
            # batch-0 attention starts before batch-1's x even lands.
            q_b = {}
            ktz = {}
            for b in range(2):
                q_b[b] = qk_pool.tile([P, S], bf16, name=f"qb{b}")
            for sc in range(2 * NQC):
                for h in range(HL):
                    ktz[h, sc] = qk_pool.tile([P, SC], bf16, name=f"ktz{h}_{sc}")
                    nc.vector.memset(ktz[h, sc][:], 0.0)

            def emit_qk(sc):
                for fi in range(2):
                    ps = at_ps_pool.tile([P, SC], f32, tag="atps", name=f"qk_ps{fi}_{sc}")
                    for e in range(NE):
                        nc.tensor.matmul(
                            ps[:],
                            wqk_sb[:, e * 2 * P + fi * P : e * 2 * P + (fi + 1) * P],
                            xt_sb[sc][:, e * SC : (e + 1) * SC],
                            start=(e == 0),
                            stop=(e == NE - 1),
                        )
                    if fi == 0:
                        # fold the 1/sqrt(D)=1/8 score scale into q (bias
                        # comes pre-scaled from the host)
                        nc.scalar.activation(
                            q_b[sc // NQC][:, (sc % NQC) * SC : (sc % NQC + 1) * SC],
                            ps[:],
                            Identity,
                            bias=bqk_sb[:, 0:1],
                            scale=0.125,
                        )
                    else:
                        for h in range(HL):
                            r0 = D * h
                            nc.scalar.activation(
                                ktz[h, sc][r0 : r0 + D, :],
                                ps[r0 : r0 + D, :],
                                Identity,
                                bias=bqk_sb[r0 : r0 + D, 1:2],
                            )

            # v quad tiles: [128, 4 si x 2 heads x 128], per-head slot
            # [v(64) | ones(1) | zeros(63)] so PV runs at M=128 with the
            # denominator free in row 64
            v_sb = {}

            def emit_v_quad(qd, pool, tag):
                psv = pool.tile([P, SC], f32, tag=tag, name=f"v_ps{qd}")
                for i in range(4):
                    si = qd * 4 + i
                    sc, j = divmod(si, 4)
                    for e in range(NE):
                        nc.tensor.matmul(
                            psv[:, i * P : (i + 1) * P],
                            xt_sb[sc][:, e * SC + j * P : e * SC + (j + 1) * P],
                            wv_sb[:, e * P : (e + 1) * P],
                            start=(e == 0),
                            stop=(e == NE - 1),
                        )
                vt = v_pool.tile([P, 4 * HL * P], bf16, name=f"vq{qd}")
                nc.vector.memset(vt[:], 0.0)
                nc.vector.memset(
                    vt[:].rearrange("p (s u) -> p s u", u=P)[:, :, D : D + 1], 1.0
                )
                nc.scalar.activation(
                    vt[:].rearrange("p (s u) -> p s u", u=P)[:, :, 0:D],
                    psv[:].rearrange("p (s u) -> p s u", u=D),
                    Identity,
                )
                v_sb[qd] = vt

            # all q,k up front (scores at kb=0 need the full q range of both
            # batches); v for b0 next; v for b1 fills PE slack in head-0's
            # key-block loop
            for sc in range(2 * NQC):
                emit_qk(sc)
            for qd in range(4):
                emit_v_quad(qd, at_ps_pool, "atps")
            # w_proj loads late into recycled b0 x-slots (dead after the
            # projections above); tile h covers c_proj blocks (h, t=0..3)
            wp_sb = {}
            for h in range(HL):
                t = xt_pool.tile([P, 4 * NX], bf16, name=f"wp{h}", tag=f"xt{h}")
                nc.sync.dma_start(t[:], wp[:, h * 4 * NX : (h + 1) * 4 * NX])
                wp_sb[h] = t

            # ---- phase 2: attention, key-block-major per (head, batch) ----
            # A2A h: slot c = head h's rows for dest core c (batch c//4,
            # chunk c%4). All 8 slots distinct - no duplication.
            a2a_in = {}
            a2a_out = {}
            for h in range(HL):
                a2a_in[h] = dram_pool.tile(
                    [8, D, SC], bf16, tag=f"a2a_in{h}", name=f"a2a_in{h}"
                )
                a2a_out[h] = dram_pool.tile(
                    [8, D, SC], bf16, tag=f"a2a_out{h}", name=f"a2a_out{h}"
                )

            def emit_tail(h, b, qc, at_ps):
                # (the v-bias is folded into bp on the host: softmax rows sum
                # to 1, so P@(v + 1 bv^T) @ wp = P@v@wp + bv@wp)
                dn32 = small_pool.tile([1, SC], f32, tag="dn32", name=f"dn{h}{b}{qc}")
                nc.vector.tensor_copy(dn32[:], at_ps[D : D + 1, :])
                rc32 = small_pool.tile([1, SC], f32, tag="rc32", name=f"rc32{h}{b}{qc}")
                nc.vector.reciprocal_approx_fast(rc32[:], dn32[:])
                rc = small_pool.tile([1, SC], f32r, tag="rc", name=f"rc{h}{b}{qc}")
                nc.vector.tensor_copy(rc[:], rc32[:])
                # broadcast 1/denom across partitions via K=1 matmul
                rb = wide_ps.tile([D, SC], f32, tag="wide", name=f"rb{h}{b}{qc}")
                nc.tensor.matmul(rb[:], ones1_sb[0:1, 0:D], rc[:], start=True, stop=True)
                rb_sb = small_pool.tile([D, SC], f32, tag="rbsb", name=f"rbsb{h}{b}{qc}")
                nc.vector.tensor_copy(rb_sb[:], rb[:])
                ath = small_pool.tile(
                    [D, SC], bf16, tag="ath", bufs=3, name=f"ath{h}{b}{qc}"
                )
                nc.vector.tensor_mul(ath[:], at_ps[0:D, :], rb_sb[:])
                nc.sync.dma_start(a2a_in[h][4 * b + qc], ath[:])

            def vslice(b, kb, h):
                si = NKB * b + kb
                return v_sb[si // 4][:, ((si % 4) * HL + h) * P :
                                     ((si % 4) * HL + h + 1) * P]

            def emit_scores(h, b, kb, s2):
                # scoresT[k, q] = ktz^T q (contraction over d, zero-padded to
                # K=128); moving pieces split at 512-aligned q-chunk
                # boundaries so each reads exactly one q tile
                q0 = P * kb
                w0 = q0 + s2 * WQ
                ww = min(WQ, S - w0)
                scp = wide_ps.tile([P, WQ], f32, tag="wide", name=f"sc{h}{b}_{kb}_{s2}")
                kt = ktz[h, NQC * b + kb // 4][:, (kb % 4) * P : (kb % 4 + 1) * P]
                for m0 in range(0, ww, SC):
                    mw = min(SC, ww - m0)
                    nc.tensor.matmul(
                        scp[:, m0 : m0 + mw],
                        kt,
                        q_b[b][:, w0 + m0 : w0 + m0 + mw],
                        start=True,
                        stop=not (s2 == 0 and m0 == 0),
                    )
                if s2 == 0:
                    # diagonal block: += causal mask via identity matmul
                    # (keeps the scores->exp chain PE-only)
                    nc.tensor.matmul(
                        scp[:, 0:P], id_sb, czb_sb, start=False, stop=True
                    )
                return scp, ww

            def emit_pv(at, h, b, kb, qc, ex, w0, ww, last):
                # one PV piece into chunk accumulator at (+denominator via
                # the ones column of v)
                a0 = max(w0, qc * SC)
                a1 = min(w0 + ww, (qc + 1) * SC)
                if a0 >= a1:
                    return
                nc.tensor.matmul(
                    at[:, a0 - qc * SC : a1 - qc * SC],
                    vslice(b, kb, h),
                    ex[:, a0 - w0 : a1 - w0],
                    start=(kb == 0),
                    stop=last,
                )

            osb = {}

            def emit_cproj(h, st, nn2):
                pp = wide_ps.tile([P, SC], f32, tag="wide", name=f"pj{h}_{st}_{nn2}")
                for t in range(4):
                    nc.tensor.matmul(
                        pp[:],
                        lh_sb[h][:, t * SC + st * P : t * SC + (st + 1) * P],
                        wp_sb[h][:, t * NX + nn2 * SC : t * NX + (nn2 + 1) * SC],
                        start=(t == 0),
                        stop=(h == 1 and t == 3),
                    )
                if h == 0:
                    # + b_proj via K=1 ones matmul
                    nc.tensor.matmul(
                        pp[:],
                        ones1b_sb[0:1, :],
                        bp_sb[0:1, nn2 * SC : (nn2 + 1) * SC],
                        start=False,
                        stop=True,
                    )
                    if nn2 == 0:
                        osb[st] = osb_pool.tile(
                            [P, NX], f32, tag=f"osb{st}", name=f"osb{st}", bufs=1
                        )
                    nc.vector.tensor_copy(osb[st][:, nn2 * SC : (nn2 + 1) * SC], pp[:])
                else:
                    nc.vector.tensor_add(
                        osb[st][:, nn2 * SC : (nn2 + 1) * SC],
                        osb[st][:, nn2 * SC : (nn2 + 1) * SC],
                        pp[:],
                    )

            # pair-fused attention per head: one key-block loop computes
            # scores+exp for BOTH batches (ScalarE exp stays saturated), with
            # batch 1 running DL key-blocks behind batch 0 (its x/qk land
            # later). PV runs live for b0; b1's ex tiles park in SBUF and its
            # per-chunk PV bursts run as pure-PE work the moment the b0
            # accumulator bank for that chunk frees - so each head's A2A
            # fires ~2us after its last exp.
            DL = 0
            lh_sb = {}
            for h in range(HL):
                at_ps = {}
                done = set()
                for qc in range(NQC):
                    at_ps[qc] = at_ps_pool.tile(
                        [P, SC], f32, tag="atps", name=f"at{h}0{qc}"
                    )
                exs = {}
                filler = [(qd, wide_ps, "wide") for qd in range(4, 8)] if h == 0 else []
                cpj = []
                for kb in range(NKB + DL):
                    if filler and kb % 2 == 1:
                        emit_v_quad(*filler.pop(0))
                    q0 = P * kb
                    for s2 in range((S - q0 + WQ - 1) // WQ):
                        w0 = q0 + s2 * WQ
                        ww = min(WQ, S - w0)
                        scp0, _ = emit_scores(h, 0, kb, s2)
                        scp1, _ = emit_scores(h, 1, kb, s2)
                        ex = exp_pool.tile(
                            [P, WQ], bf16, tag="exp", name=f"ex{h}_{kb}_{s2}"
                        )
                        nc.scalar.activation(ex[:, 0:ww], scp0[:, 0:ww], Exp)
                        ext = exs_pool.tile(
                            [P, ww], bf16, tag=f"exs{kb}_{s2}",
                            name=f"exs{h}_{kb}_{s2}",
                        )
                        nc.scalar.activation(ext[:], scp1[:, 0:ww], Exp)
                        exs[kb, s2] = ext
                        for qc in range(w0 // SC, (w0 + ww - 1) // SC + 1):
                            emit_pv(at_ps[qc], h, 0, kb, qc, ex, w0, ww,
                                    kb == 4 * qc + 3)
                    # deferred normalization: ~2 key-blocks after a chunk's
                    # last contribution, so the PE rarely waits on the DVE
                    # reciprocal chain
                    for qc in range(NQC):
                        if kb == min(4 * qc + 5, NKB + DL - 1) and qc not in done:
                            done.add(qc)
                            emit_tail(h, 0, qc, at_ps[qc])
                    # b1 PV burst, chunk-major from the parked ex tiles;
                    # bursts ride the late-kb PE slack once the b0
                    # accumulator bank for that chunk has freed
                    bqc = (kb - 6) // 4
                    if kb >= 6 and (kb - 6) % 4 == 0 and bqc < 3:
                        at2 = at_ps_pool.tile(
                            [P, SC], f32, tag="atps", name=f"at{h}1{bqc}"
                        )
                        for kbb in range(4 * bqc + 4):
                            qq0 = P * kbb
                            for s2 in range((S - qq0 + WQ - 1) // WQ):
                                w0 = qq0 + s2 * WQ
                                ww = min(WQ, S - w0)
                                emit_pv(at2, h, 1, kbb, bqc, exs[kbb, s2], w0, ww,
                                        kbb == 4 * bqc + 3)
                        emit_tail(h, 1, bqc, at2)
                # last chunk's b1 burst right after the loop
                at2 = at_ps_pool.tile([P, SC], f32, tag="atps", name=f"at{h}13")
                for kbb in range(NKB):
                    qq0 = P * kbb
                    for s2 in range((S - qq0 + WQ - 1) // WQ):
                        w0 = qq0 + s2 * WQ
                        ww = min(WQ, S - w0)
                        emit_pv(at2, h, 1, kbb, 3, exs[kbb, s2], w0, ww,
                                kbb == NKB - 1)
                emit_tail(h, 1, 3, at2)
                nc.gpsimd.collective_compute(
                    "AllToAll",
                    mybir.AluOpType.bypass,
                    ins=[a2a_in[h][:].opt()],
                    outs=[a2a_out[h][:].opt()],
                    replica_groups=[list(range(8))],
                )
                # gather: [128 rows = src pair (2t, 2t+1) stacked,
                # 4 x 512 cols]; wp host-permuted to match block (h, t)
                lht = lh_pool.tile([P, 4 * SC], bf16, name=f"lh{h}")
                for u in range(2):
                    nc.sync.dma_start(
                        lht[u * D : (u + 1) * D, :],
                        a2a_out[h][:].rearrange("(t u) p j -> u p t j", u=2)[u],
                    )
                lh_sb[h] = lht

            # ---- phase 3: c_proj both passes ----
            for st in range(4):
                for nn2 in range(2):
                    emit_cproj(0, st, nn2)
            for st in range(4):
                for nn2 in range(2):
                    emit_cproj(1, st, nn2)
                nc.sync.dma_start(out_ext[st * P : (st + 1) * P, :], osb[st][:])

    nc.compile()
    return nc


def _get_compiled():
    global _COMPILED
    if _COMPILED is None:
        _COMPILED = _build()
    return _COMPILED


def make_in_maps(x, attention_mask, w_attn, b_attn, w_proj, b_proj):
    x = np.asarray(x, dtype=np.float32)
    w_attn = np.asarray(w_attn, dtype=np.float32)
    b_attn = np.asarray(b_attn, dtype=np.float32)
    w_proj = np.asarray(w_proj, dtype=np.float32)
    b_proj = np.asarray(b_proj, dtype=np.float32)

    ki, qi = np.meshgrid(np.arange(P), np.arange(P), indexing="ij")
    causalT = np.where(ki > qi, np.float32(-1e9), np.float32(0.0))
    idcz = np.concatenate([np.eye(P, dtype=BF16), causalT.astype(BF16)], axis=1)

    # x pre-tiled: xtt[b*4+sc][p, e*512+j] = x[b, sc*512+j, e*128+p]
    xtt = np.ascontiguousarray(
        x.reshape(B, NQC, SC, NE, P).transpose(0, 1, 4, 3, 2).reshape(
            B * NQC, P, NE * SC
        ).astype(BF16)
    )

    # wp permuted: block (h, t) rows = [head 4t+h | head 4t+2+h] (64 each)
    blocks = []
    for h in range(2):
        for t in range(4):
            blocks.append(w_proj[(4 * t + h) * D : (4 * t + h) * D + D, :])
            blocks.append(w_proj[(4 * t + 2 + h) * D : (4 * t + 2 + h) * D + D, :])
    wp_perm = np.concatenate(blocks, axis=0)  # [1024, 1024]
    wp_tiled = np.ascontiguousarray(
        wp_perm.reshape(8, P, NX).transpose(1, 0, 2).reshape(P, 8 * NX).astype(BF16)
    )

    bv_full = b_attn[2 * NX : 3 * NX].astype(np.float64)
    bp_eff = (b_proj.astype(np.float64) + bv_full @ w_proj.astype(np.float64)).astype(
        np.float32
    )
    bp_row16 = np.ascontiguousarray(bp_eff.reshape(1, NX).astype(BF16))

    in_maps = []
    for s in range(8):
        qcols = slice(P * s, P * s + P)
        kcols = slice(NX + P * s, NX + P * s + P)
        vcols = slice(2 * NX + P * s, 2 * NX + P * s + P)
        # wqk interleaved: [p, e*256 + fi*128 + f] = w_attn[e*128+p, (q|k)col f]
        wq = w_attn[:, qcols].reshape(NE, P, P)
        wk = w_attn[:, kcols].reshape(NE, P, P)
        wqk_t = np.ascontiguousarray(
            np.stack([wq, wk], axis=1)  # [e, fi, p, f]
            .transpose(2, 0, 1, 3)
            .reshape(P, NE * 2 * P)
            .astype(BF16)
        )
        wv_t = np.ascontiguousarray(
            w_attn[:, vcols].reshape(NE, P, P).transpose(1, 0, 2).reshape(P, NE * P)
            .astype(BF16)
        )
        bqk_arr = np.stack(
            [b_attn[qcols] * 0.125, b_attn[kcols]], axis=1
        ).astype(np.float32)
        in_maps.append(
            {
                "xtt": xtt,
                "wqk": wqk_t,
                "wv": wv_t,
                "wp": wp_tiled,
                "bqk": np.ascontiguousarray(bqk_arr),
                "bp16": bp_row16,
                "idcz": idcz,
                "ones1": np.ones((1, P), dtype=np.float32),
                "ones1b": np.ones((1, P), dtype=BF16),
            }
        )
    return in_maps


def assemble_out(results):
    out = np.empty((B, S, NX), dtype=np.float32)
    for c in range(8):
        b, g = divmod(c, 4)
        out[b, g * SC : (g + 1) * SC, :] = results[c]["out"]
    return out


def run(in_maps, trace=False):
    from concourse.bass_utils import run_bass_kernel_spmd

    nc = _get_compiled()
    return run_bass_kernel_spmd(nc, in_maps, core_ids=list(range(8)), trace=trace)


def kernel(**inputs) -> np.ndarray:
    in_maps = make_in_maps(**inputs)
    res = run(in_maps)
    return assemble_out(res.results)


if __name__ == "__main__":
    _get_compiled()
    print("build+compile OK")
